# revision 1
# baseline (speedup 1.0000x reference)
"""Causal self-attention (B=4, T=2048, C=1024, 16 heads, interleaved RoPE)
on 8 trn2 NeuronCores.

Sharding: tensor-parallel over heads -- core c owns heads {2c, 2c+1} (128
channels). Each core:
  - loads full x, transposes it on the PE to x^T tiles (fp32 exact),
  - projects Q^T/K^T/V^T [128=2*64, T] per batch in f32r (full-rate matmuls),
  - applies interleaved RoPE as rope(p) = p*cos + Pswap @ (p*sin_signed),
    with Pswap (the even/odd swap permutation) one PE matmul,
  - scores S^T[kv,q] = K Q^T per head in double-wide [128,1024] PSUM tiles
    (pairs of kv blocks -> half the ACT exp instructions); exp on ACT
    (scale=1/8; no max subtraction needed, scores are ~N(0,1)); causal
    handled by column sub-ranges + a triangular multiplicative mask,
  - y^T = V_aug^T @ P^T where V_aug carries a ones column, fusing softmax
    row-sums into the PV matmul; normalization uses a DMA lane-spread
    reciprocal + gpsimd partition broadcast,
  - partial output projection y^T_c @ wo_c -> [B, T, C] partial.
Host sums the 8 partials (the all-reduce of the hinted TP scheme).

The emission is software-pipelined: stage-1 tiles of batch b+1 are emitted
between attention q-tiles of batch b so the statically-scheduled PE stream
has fill work during softmax/normalize dependency stalls.

Self-contained: hardcoded shapes, no reads of /root/problem/*.
"""
import numpy as np

import concourse.bacc as bacc
import concourse.mybir as mybir
import concourse.tile as tile
from concourse.bass_utils import run_bass_kernel_spmd
from concourse.masks import make_identity, make_upper_triangular

B, T, C = 4, 2048, 1024
NH, D = 16, 64
NCORES = 8
HL = NH // NCORES  # heads per core
HD = HL * D  # per-core head channels = 128
QTILE = 512
KB = T // 128  # kv blocks per batch = 16
NJ = T // QTILE  # q tiles per batch = 4
CB = C // 128  # channel blocks = 8
F32 = mybir.dt.float32
F32R = mybir.dt.float32r
EXP = mybir.ActivationFunctionType.Exp

_CACHE = {}


def build(num_batches=B):
    nc = bacc.Bacc(None, target_bir_lowering=False)
    x_d = nc.declare_dram_parameter("x", [B, T, C], F32, isOutput=False)
    wq_d = nc.declare_dram_parameter("wqt", [C, HD], F32, isOutput=False)
    wk_d = nc.declare_dram_parameter("wkt", [C, HD], F32, isOutput=False)
    wv_d = nc.declare_dram_parameter("wvt", [C, HD], F32, isOutput=False)
    wo_d = nc.declare_dram_parameter("wot", [HD, C], F32, isOutput=False)
    cos_d = nc.declare_dram_parameter("cosb", [HD, T], F32, isOutput=False)
    sin_d = nc.declare_dram_parameter("sinb", [HD, T], F32, isOutput=False)
    psw_d = nc.declare_dram_parameter("pswap", [128, 128], F32, isOutput=False)
    out_d = nc.declare_dram_parameter("out", [B, T, C], F32, isOutput=True)

    with tile.TileContext(nc) as tc:
        with (
            tc.tile_pool(name="const", bufs=1) as const,
            tc.tile_pool(name="wpool", bufs=1) as wpool,
            tc.tile_pool(name="xsb", bufs=2) as xsb,
            tc.tile_pool(name="xtp", bufs=1) as xtp,
            tc.tile_pool(name="qkp", bufs=2) as qkp,
            tc.tile_pool(name="vtp", bufs=2) as vtp,
            tc.tile_pool(name="vap", bufs=2) as vap,
            tc.tile_pool(name="ytp", bufs=2) as ytp,
            tc.tile_pool(name="ptp", bufs=5) as ptp,
            tc.tile_pool(name="yup", bufs=4) as yup,
            tc.tile_pool(name="npool", bufs=2) as npool,
            tc.tile_pool(name="opool", bufs=3) as opool,
            tc.tile_pool(name="ps", bufs=2, space="PSUM") as ps,
        ):
            # ---- constants ----
            ident = const.tile([128, 128], F32)
            make_identity(nc, ident)
            tri_f = const.tile([128, 128], F32)
            make_upper_triangular(nc, tri_f, val=1.0, diag=True)  # 1 if i<=j
            tri = const.tile([128, 128], F32R)
            nc.vector.tensor_copy(tri[:], tri_f[:])
            zeros = const.tile([128, 128], F32R)
            nc.gpsimd.memset(zeros.bitcast(F32)[:], 0.0)
            cos_t = const.tile([HD, T], F32)
            sin_t = const.tile([HD, T], F32)
            nc.sync.dma_start(out=cos_t[:], in_=cos_d[:])
            nc.sync.dma_start(out=sin_t[:], in_=sin_d[:])

            # ---- weights -> f32r ----
            with tc.tile_pool(name="wstage", bufs=1) as wstage:
                def load_kxm(dram, name):
                    stg = wstage.tile([128, CB, HD], F32, name=f"{name}_f", tag="wst")
                    nc.sync.dma_start(
                        out=stg[:], in_=dram.ap().rearrange("(cb p) m -> p cb m", p=128)
                    )
                    wr = wpool.tile([128, CB, HD], F32R, name=f"{name}_r")
                    nc.vector.tensor_copy(wr[:], stg[:])
                    return wr

                wq_r = load_kxm(wq_d, "wq")
                wk_r = load_kxm(wk_d, "wk")
                wv_r = load_kxm(wv_d, "wv")
                wo_f = wstage.tile([HD, C], F32, tag="wst2")
                nc.sync.dma_start(out=wo_f[:], in_=wo_d[:])
                wo_r = wpool.tile([HD, C], F32R)
                nc.vector.tensor_copy(wo_r[:], wo_f[:])
                psw_f = wstage.tile([128, 128], F32, tag="wst3")
                nc.sync.dma_start(out=psw_f[:], in_=psw_d[:])
                psw_r = wpool.tile([128, 128], F32R)
                nc.vector.tensor_copy(psw_r[:], psw_f[:])

            bt = {}  # per-batch live tensors

            def stage1_tile(b, tt):
                """Transpose + project + rope one 512-token tile of batch b."""
                if tt == 0:
                    s = bt[b] = {
                        "qt": qkp.tile([HD, T], F32R, name=f"qt_{b}", tag="qt"),
                        "kt": qkp.tile([HD, T], F32R, name=f"kt_{b}", tag="kt"),
                        "vaa": vap.tile([128, KB, D + 1], F32R, name=f"vaa_{b}", tag="va_a"),
                        "vab": vap.tile([128, KB, D + 1], F32R, name=f"vab_{b}", tag="va_b"),
                        "yt": ytp.tile([HD, T], F32R, name=f"yt_{b}", tag="yt_b"),
                    }
                    nc.gpsimd.memset(s["vaa"].bitcast(F32)[:, :, D : D + 1], 1.0)
                    nc.gpsimd.memset(s["vab"].bitcast(F32)[:, :, D : D + 1], 1.0)
                s = bt[b]
                t0 = tt * QTILE
                x_sb0 = xsb.tile([128, 2, C], F32, name="x_sb0", tag="xsb")
                nc.sync.dma_start(
                    out=x_sb0[:],
                    in_=x_d.ap()[b, t0 : t0 + 256, :].rearrange("(ts p) c -> p ts c", p=128),
                )
                x_sb1 = xsb.tile([128, 2, C], F32, name="x_sb1", tag="xsb")
                nc.sync.dma_start(
                    out=x_sb1[:],
                    in_=x_d.ap()[b, t0 + 256 : t0 + QTILE, :].rearrange(
                        "(ts p) c -> p ts c", p=128
                    ),
                )
                xt_t = xtp.tile([128, CB, QTILE], F32R, name="xt_t")
                for cb in range(CB):
                    xt_ps = ps.tile([128, QTILE], F32, name="xt_ps", tag="s1", bufs=3)
                    cs = slice(cb * 128, (cb + 1) * 128)
                    nc.tensor.transpose(xt_ps[:, 0:128], x_sb0[:, 0, cs], ident[:])
                    nc.tensor.transpose(xt_ps[:, 128:256], x_sb0[:, 1, cs], ident[:])
                    nc.tensor.transpose(xt_ps[:, 256:384], x_sb1[:, 0, cs], ident[:])
                    nc.tensor.transpose(xt_ps[:, 384:512], x_sb1[:, 1, cs], ident[:])
                    nc.vector.tensor_copy(xt_t[:, cb, :], xt_ps[:])

                # q/k projections with fused rope
                for wname, wr, dst in (("q", wq_r, s["qt"]), ("k", wk_r, s["kt"])):
                    pj = ps.tile([HD, QTILE], F32, name=f"pj_{wname}", tag="s1", bufs=3)
                    for cb in range(CB):
                        nc.tensor.matmul(
                            pj[:], wr[:, cb, :], xt_t[:, cb, :],
                            start=(cb == 0), stop=(cb == CB - 1),
                        )
                    pjs = npool.tile([HD, QTILE], F32R, name="pjs", tag="pjs")
                    nc.vector.tensor_mul(pjs[:], pj[:], sin_t[:, t0 : t0 + QTILE])
                    pjc = npool.tile([HD, QTILE], F32, name="pjc", tag="pjc")
                    nc.vector.tensor_mul(pjc[:], pj[:], cos_t[:, t0 : t0 + QTILE])
                    rope_ps = ps.tile([HD, QTILE], F32, name="rope_ps", tag="s1", bufs=3)
                    nc.tensor.matmul(rope_ps[:], psw_r[:], pjs[:], start=True, stop=True)
                    nc.vector.tensor_add(dst[:, t0 : t0 + QTILE], rope_ps[:], pjc[:])

                # v projection -> natural layout with ones column
                pj = ps.tile([HD, QTILE], F32, name="pj_v", tag="s1", bufs=3)
                for cb in range(CB):
                    nc.tensor.matmul(
                        pj[:], wv_r[:, cb, :], xt_t[:, cb, :],
                        start=(cb == 0), stop=(cb == CB - 1),
                    )
                vt_sb = vtp.tile([HD, QTILE], F32, name="vt_sb")
                nc.vector.tensor_copy(vt_sb[:], pj[:])
                for kvt in range(4):
                    kv = tt * 4 + kvt
                    c128 = slice(kvt * 128, (kvt + 1) * 128)
                    vtr = ps.tile([128, 128], F32, name="vtr", tag="s1", bufs=3)
                    nc.tensor.transpose(vtr[:], vt_sb[:, c128], ident[:])
                    nc.vector.tensor_copy(s["vaa"][:, kv, 0:D], vtr[:, 0:64])
                    nc.vector.tensor_copy(s["vab"][:, kv, 0:D], vtr[:, 64:128])

            def attention(b, j):
                s = bt[b]
                q0 = j * QTILE
                for h in range(HL):
                    hp = h * D
                    va = s["vaa"] if h == 0 else s["vab"]
                    yt_ps = ps.tile([D + 1, QTILE], F32, name="yt_ps", tag="yt", bufs=1)
                    nblk = 4 * (j + 1)
                    # kv blocks in pairs on a double-wide score tile
                    for pr in range(nblk // 2):
                        halves = []
                        st = ps.tile([128, 2 * QTILE], F32, name="st", tag="st")
                        pt = ptp.tile([128, 2 * QTILE], F32R, name="pt", bufs=5)
                        for idx in range(2):
                            k = 2 * pr + idx
                            m = k - 4 * j
                            c0 = 0 if m < 0 else min(m, 2) * 128
                            e0 = 0 if m < 0 else m * 128
                            halves.append((idx, k, m, c0, e0))
                            nc.tensor.matmul(
                                st[:, idx * QTILE + c0 : (idx + 1) * QTILE],
                                s["kt"][hp : hp + D, k * 128 : (k + 1) * 128],
                                s["qt"][hp : hp + D, q0 + c0 : q0 + QTILE],
                                start=True, stop=True,
                            )
                        if halves[0][2] < 0:  # full pair: one wide exp
                            nc.scalar.activation(pt[:], st[:], EXP, scale=0.125)
                        else:
                            for idx, k, m, c0, e0 in halves:
                                o = idx * QTILE
                                nc.scalar.activation(
                                    pt[:, o + e0 : o + QTILE],
                                    st[:, o + e0 : o + QTILE],
                                    EXP, scale=0.125,
                                )
                                nc.vector.tensor_mul(
                                    pt[:, o + e0 : o + e0 + 128],
                                    pt[:, o + e0 : o + e0 + 128],
                                    tri[:],
                                )
                                if e0 > c0:
                                    nc.vector.tensor_copy(
                                        pt[:, o + c0 : o + e0], zeros[:, 0 : e0 - c0]
                                    )
                        for idx, k, m, c0, e0 in halves:
                            nc.tensor.matmul(
                                yt_ps[:, c0:QTILE],
                                va[:, k, :],
                                pt[:, idx * QTILE + c0 : (idx + 1) * QTILE],
                                start=(k == 0), stop=(k == nblk - 1),
                            )
                    # stash y^T + sums to SBUF, freeing the PSUM bank fast
                    yu = yup.tile([D + 1, QTILE], F32, name="yu")
                    nc.vector.tensor_copy(yu[:], yt_ps[:])
                    # reciprocal of sums, lane-spread via DMA reshape [1,512]->[128,4]
                    s128 = npool.tile([128, 4], F32, name="s128", tag="s128", bufs=4)
                    nc.sync.dma_start(out=s128[:], in_=yu[D : D + 1, :])
                    r128 = npool.tile([128, 4], F32, name="r128", tag="r128", bufs=4)
                    nc.vector.reciprocal(r128[:], s128[:])
                    rrow = npool.tile([1, QTILE], F32, name="rrow", tag="rrow", bufs=4)
                    nc.sync.dma_start(out=rrow[:], in_=r128[:])
                    rbc = npool.tile([D, QTILE], F32, name="rbc", tag="rbc", bufs=3)
                    nc.gpsimd.partition_broadcast(rbc[:], rrow[:])
                    nc.vector.tensor_mul(
                        s["yt"][hp : hp + D, q0 : q0 + QTILE], yu[0:D, :], rbc[:]
                    )

            def outproj(b, jo):
                s = bt[b]
                for tb in range(4 * jo, 4 * (jo + 1)):
                    for co in range(C // QTILE):
                        op = ps.tile([128, QTILE], F32, name="op", tag="s1", bufs=3)
                        nc.tensor.matmul(
                            op[:],
                            s["yt"][:, tb * 128 : (tb + 1) * 128],
                            wo_r[:, co * QTILE : (co + 1) * QTILE],
                            start=True, stop=True,
                        )
                        ot = opool.tile([128, QTILE], F32, name="ot")
                        nc.scalar.copy(ot[:], op[:])
                        nc.sync.dma_start(
                            out=out_d.ap()[
                                b, tb * 128 : (tb + 1) * 128,
                                co * QTILE : (co + 1) * QTILE,
                            ],
                            in_=ot[:],
                        )

            # ---- software-pipelined emission ----
            for tt in range(NJ):
                stage1_tile(0, tt)
            for b in range(num_batches):
                for j in range(NJ):
                    attention(b, j)
                    if b + 1 < num_batches:
                        stage1_tile(b + 1, j)
                    if j > 0:
                        outproj(b, j - 1)
                outproj(b, NJ - 1)
    nc.finalize()
    return nc


def _rope_tables():
    freqs = 1.0 / (10000.0 ** (np.arange(0, D, 2, dtype=np.float64) / D))  # [32]
    grid = np.arange(T, dtype=np.float64)[:, None] * freqs[None, :]  # [T, 32]
    cos = np.cos(grid)
    sin = np.sin(grid)
    # row d uses freq d//2; sin sign: + for even d, - for odd d
    cos_b = np.repeat(cos.T, 2, axis=0)  # [64, T]
    sin_b = np.repeat(sin.T, 2, axis=0)
    sin_b[1::2] *= -1.0
    cos_hd = np.tile(cos_b, (HL, 1)).astype(np.float32)  # [128, T]
    sin_hd = np.tile(sin_b, (HL, 1)).astype(np.float32)
    return np.ascontiguousarray(cos_hd), np.ascontiguousarray(sin_hd)


def _pswap():
    p = np.zeros((128, 128), dtype=np.float32)
    idx = np.arange(0, 128, 2)
    p[idx, idx + 1] = 1.0
    p[idx + 1, idx] = 1.0
    return p


def kernel(x, wq, wk, wv, wo):
    if "nc" not in _CACHE:
        _CACHE["nc"] = build()
    nc = _CACHE["nc"]

    cos_hd, sin_hd = _rope_tables()
    psw = _pswap()
    x = np.ascontiguousarray(x, dtype=np.float32)
    core_ids = list(range(NCORES))
    in_maps = []
    for c in core_ids:
        r0 = c * HD
        in_maps.append(
            {
                "x": x,
                "wqt": np.ascontiguousarray(wq[r0 : r0 + HD, :].T),
                "wkt": np.ascontiguousarray(wk[r0 : r0 + HD, :].T),
                "wvt": np.ascontiguousarray(wv[r0 : r0 + HD, :].T),
                "wot": np.ascontiguousarray(wo[:, r0 : r0 + HD].T),
                "cosb": cos_hd,
                "sinb": sin_hd,
                "pswap": psw,
            }
        )
    try:
        res = run_bass_kernel_spmd(nc, in_maps, core_ids).results
    except Exception:
        # transient NRT/device hiccup: retry once
        res = run_bass_kernel_spmd(nc, in_maps, core_ids).results
    out = np.zeros((B, T, C), dtype=np.float32)
    for c in core_ids:
        out += res[c]["out"]
    return out



# revision 2
# speedup vs baseline: 1.5975x; 1.5975x over previous
"""Causal self-attention (B=4, T=2048, C=1024, 16 heads, interleaved RoPE)
on 8 trn2 NeuronCores.

Sharding: 4x2 grid (batch x head-half). Core c owns batch c//2 and heads
(c%2)*8 .. (c%2)*8+7 (512 head channels). Each core loads only its batch's
x slice (bf16), projects Q/K/V for its 8 heads, runs causal attention, and
produces a partial [T, C] output via its wo slice; the host sums the two
partials per batch (the all-reduce of the hinted TP scheme).

All matmul inputs are bf16 (weights/x/tables cast on host): 2-byte moving
operands stream the PE at full rate and halve SBUF/PSUM/DMA traffic vs
f32r; accumulation stays fp32 in PSUM, softmax normalization in fp32.
RoPE is applied as rope(p) = p*cos + Pswap @ (p*sin_signed) with Pswap the
even/odd swap permutation (one PE matmul). Scores S^T[kv, q] = K Q^T per
head in double-wide [128, 1024] PSUM tiles (pairs of kv blocks -> half the
ACT exp instructions); exp on ACT with scale=1/8 (no max subtraction:
scores ~N(0,1)); causality via per-block column sub-ranges + a triangular
multiplicative mask on the diagonal blocks. y^T = V_aug^T @ P^T with a
ones-column in V_aug fusing the softmax row-sums into the PV matmul;
normalization uses a DMA lane-spread reciprocal + gpsimd partition
broadcast. V is projected directly into natural [t, hd] layout (x^T tiles
as the stationary operand), which removes the separate V transposes.

Self-contained: hardcoded shapes, no reads of /root/problem/*.
"""
import numpy as np
import ml_dtypes

import concourse.bacc as bacc
import concourse.mybir as mybir
import concourse.tile as tile
from concourse.bass_utils import run_bass_kernel_spmd
from concourse.masks import make_identity, make_upper_triangular

B, T, C = 4, 2048, 1024
NH, D = 16, 64
NCORES = 8
NHL = 8  # heads per core
HD = NHL * D  # per-core head channels = 512
HDB = HD // 128  # head-dim partition blocks = 4
QTILE = 512
KB = T // 128  # kv blocks = 16
NJ = T // QTILE  # q tiles = 4
CB = C // 128  # channel blocks = 8
F32 = mybir.dt.float32
BF16 = mybir.dt.bfloat16
EXP = mybir.ActivationFunctionType.Exp
BF = ml_dtypes.bfloat16

_CACHE = {}


def build():
    nc = bacc.Bacc(None, target_bir_lowering=False)
    x_d = nc.declare_dram_parameter("x", [T, C], BF16, isOutput=False)
    wq_d = nc.declare_dram_parameter("wqt", [C, HD], BF16, isOutput=False)
    wk_d = nc.declare_dram_parameter("wkt", [C, HD], BF16, isOutput=False)
    wv_d = nc.declare_dram_parameter("wvt", [C, HD], BF16, isOutput=False)
    wo_d = nc.declare_dram_parameter("wot", [HD, C], BF16, isOutput=False)
    cos_d = nc.declare_dram_parameter("cosb", [128, T], F32, isOutput=False)
    sin_d = nc.declare_dram_parameter("sinb", [128, T], F32, isOutput=False)
    psw_d = nc.declare_dram_parameter("pswap", [128, 128], BF16, isOutput=False)
    out_d = nc.declare_dram_parameter("out", [T, C], F32, isOutput=True)

    with tile.TileContext(nc) as tc:
        with (
            tc.tile_pool(name="const", bufs=1) as const,
            tc.tile_pool(name="wpool", bufs=1) as wpool,
            tc.tile_pool(name="xsb", bufs=2) as xsb,
            tc.tile_pool(name="xtp", bufs=2) as xtp,
            tc.tile_pool(name="qkp", bufs=1) as qkp,
            tc.tile_pool(name="vap", bufs=1) as vap,
            tc.tile_pool(name="ytp", bufs=1) as ytp,
            tc.tile_pool(name="ptp", bufs=4) as ptp,
            tc.tile_pool(name="yup", bufs=4) as yup,
            tc.tile_pool(name="npool", bufs=2) as npool,
            tc.tile_pool(name="opool", bufs=3) as opool,
            tc.tile_pool(name="ps", bufs=2, space="PSUM") as ps,
        ):
            # ---- constants ----
            ident_f = const.tile([128, 128], F32)
            make_identity(nc, ident_f)
            ident = const.tile([128, 128], BF16)
            nc.vector.tensor_copy(ident[:], ident_f[:])
            tri_f = const.tile([128, 128], F32)
            make_upper_triangular(nc, tri_f, val=1.0, diag=True)  # 1 if i<=j
            tri = const.tile([128, 128], BF16)
            nc.vector.tensor_copy(tri[:], tri_f[:])
            cos_t = const.tile([128, T], F32)
            sin_t = const.tile([128, T], F32)
            nc.sync.dma_start(out=cos_t[:], in_=cos_d[:])
            nc.sync.dma_start(out=sin_t[:], in_=sin_d[:])
            psw = const.tile([128, 128], BF16)
            nc.sync.dma_start(out=psw[:], in_=psw_d[:])

            # ---- weights (bf16 straight from DRAM) ----
            wq_b = wpool.tile([128, CB, HD], BF16)
            wk_b = wpool.tile([128, CB, HD], BF16)
            wv_b = wpool.tile([128, CB, HD], BF16)
            for dram, sb in ((wq_d, wq_b), (wk_d, wk_b), (wv_d, wv_b)):
                nc.sync.dma_start(
                    out=sb[:], in_=dram.ap().rearrange("(cb p) m -> p cb m", p=128)
                )
            wo_b = wpool.tile([128, HDB, C], BF16)
            nc.sync.dma_start(
                out=wo_b[:], in_=wo_d.ap().rearrange("(hb p) c -> p hb c", p=128)
            )

            # ---- persistent per-batch tensors ----
            qt = [qkp.tile([128, T], BF16, name=f"qt{hb}") for hb in range(HDB)]
            kt = [qkp.tile([128, T], BF16, name=f"kt{hb}") for hb in range(HDB)]
            yt = [ytp.tile([128, T], BF16, name=f"yt{hb}") for hb in range(HDB)]
            va = vap.tile([128, KB, NHL, D + 1], BF16)
            nc.gpsimd.memset(va[:, :, :, D : D + 1], 1.0)

            def stage1(tt):
                """Transpose + project + rope one 512-token tile."""
                t0 = tt * QTILE
                x_sb = xsb.tile([128, 4, C], BF16, name="x_sb", tag="xsb")
                nc.sync.dma_start(
                    out=x_sb[:],
                    in_=x_d.ap()[t0 : t0 + QTILE, :].rearrange(
                        "(ts p) c -> p ts c", p=128
                    ),
                )
                xt = xtp.tile([128, CB, QTILE], BF16, name="xt", tag="xt")
                for cb in range(CB):
                    xt_ps = ps.tile([128, QTILE], BF16, name="xt_ps", tag="s1", bufs=2)
                    cs = slice(cb * 128, (cb + 1) * 128)
                    for sub in range(4):
                        nc.tensor.transpose(
                            xt_ps[:, sub * 128 : (sub + 1) * 128],
                            x_sb[:, sub, cs],
                            ident[:],
                        )
                    nc.vector.tensor_copy(xt[:, cb, :], xt_ps[:])

                # q/k projections with fused rope, per head-dim block
                for wr, dst in ((wq_b, qt), (wk_b, kt)):
                    for hb in range(HDB):
                        hs = slice(hb * 128, (hb + 1) * 128)
                        pj = ps.tile([128, QTILE], F32, name="pj", tag="s1", bufs=2)
                        for cb in range(CB):
                            nc.tensor.matmul(
                                pj[:], wr[:, cb, hs], xt[:, cb, :],
                                start=(cb == 0), stop=(cb == CB - 1),
                            )
                        pjs = npool.tile([128, QTILE], BF16, name="pjs", tag="pjs")
                        nc.vector.tensor_mul(pjs[:], pj[:], sin_t[:, t0 : t0 + QTILE])
                        pjc = npool.tile([128, QTILE], F32, name="pjc", tag="pjc")
                        nc.vector.tensor_mul(pjc[:], pj[:], cos_t[:, t0 : t0 + QTILE])
                        rps = ps.tile([128, QTILE], F32, name="rps", tag="s1", bufs=2)
                        nc.tensor.matmul(rps[:], psw[:], pjs[:], start=True, stop=True)
                        nc.vector.tensor_add(
                            dst[hb][:, t0 : t0 + QTILE], rps[:], pjc[:]
                        )

                # v projection straight into natural [t, hd] layout:
                # stationary = x^T tile, moving = wv row-block
                for tb in range(4):
                    vj = ps.tile([128, HD], F32, name="vj", tag="s1", bufs=2)
                    ts = slice(tb * 128, (tb + 1) * 128)
                    for cb in range(CB):
                        nc.tensor.matmul(
                            vj[:], xt[:, cb, ts], wv_b[:, cb, :],
                            start=(cb == 0), stop=(cb == CB - 1),
                        )
                    kv = tt * 4 + tb
                    nc.vector.tensor_copy(
                        va[:, kv, :, 0:D],
                        vj[:].rearrange("p (h d) -> p h d", h=NHL),
                    )

            def attention(j):
                q0 = j * QTILE
                nblk = 4 * (j + 1)
                for h in range(NHL):
                    hb, hp = h // 2, (h % 2) * D
                    yt_ps = ps.tile([D + 1, QTILE], F32, name="yt_ps", tag="yt", bufs=2)
                    for pr in range(nblk // 2):
                        st = ps.tile([128, 2 * QTILE], F32, name="st", tag="st", bufs=2)
                        pt = ptp.tile([128, 2 * QTILE], BF16, name="pt", bufs=4)
                        halves = []
                        for idx in range(2):
                            k = 2 * pr + idx
                            m = k - 4 * j
                            e0 = 0 if m < 0 else m * 128
                            halves.append((idx, k, m, e0))
                            nc.tensor.matmul(
                                st[:, idx * QTILE + e0 : (idx + 1) * QTILE],
                                kt[hb][hp : hp + D, k * 128 : (k + 1) * 128],
                                qt[hb][hp : hp + D, q0 + e0 : q0 + QTILE],
                                start=True, stop=True,
                            )
                        if halves[0][2] < 0:  # full pair: one wide exp
                            nc.scalar.activation(pt[:], st[:], EXP, scale=0.125)
                        else:
                            for idx, k, m, e0 in halves:
                                o = idx * QTILE
                                nc.scalar.activation(
                                    pt[:, o + e0 : o + QTILE],
                                    st[:, o + e0 : o + QTILE],
                                    EXP, scale=0.125,
                                )
                                nc.vector.tensor_mul(
                                    pt[:, o + e0 : o + e0 + 128],
                                    pt[:, o + e0 : o + e0 + 128],
                                    tri[:],
                                )
                        for idx, k, m, e0 in halves:
                            nc.tensor.matmul(
                                yt_ps[:, e0:QTILE],
                                va[:, k, h, :],
                                pt[:, idx * QTILE + e0 : (idx + 1) * QTILE],
                                start=(k == 0), stop=(k == nblk - 1),
                            )
                    # softmax normalization: row sums live in partition D
                    yu = yup.tile([D + 1, QTILE], F32, name="yu")
                    nc.vector.tensor_copy(yu[:], yt_ps[:])
                    s128 = npool.tile([128, 4], F32, name="s128", tag="s128", bufs=4)
                    nc.sync.dma_start(out=s128[:], in_=yu[D : D + 1, :])
                    r128 = npool.tile([128, 4], F32, name="r128", tag="r128", bufs=4)
                    nc.vector.reciprocal(r128[:], s128[:])
                    rrow = npool.tile([1, QTILE], F32, name="rrow", tag="rrow", bufs=4)
                    nc.sync.dma_start(out=rrow[:], in_=r128[:])
                    rbc = npool.tile([D, QTILE], F32, name="rbc", tag="rbc", bufs=3)
                    nc.gpsimd.partition_broadcast(rbc[:], rrow[:])
                    nc.vector.tensor_mul(
                        yt[hb][hp : hp + D, q0 : q0 + QTILE], yu[0:D, :], rbc[:]
                    )

            def outproj(jo):
                for tb in range(4 * jo, 4 * (jo + 1)):
                    ts = slice(tb * 128, (tb + 1) * 128)
                    for co in range(C // QTILE):
                        op = ps.tile([128, QTILE], F32, name="op", tag="s1", bufs=2)
                        for hb in range(HDB):
                            nc.tensor.matmul(
                                op[:],
                                yt[hb][:, ts],
                                wo_b[:, hb, co * QTILE : (co + 1) * QTILE],
                                start=(hb == 0), stop=(hb == HDB - 1),
                            )
                        ot = opool.tile([128, QTILE], F32, name="ot")
                        # split PSUM->SBUF copies between ACT and DVE
                        if (tb + co) % 2 == 0:
                            nc.scalar.copy(ot[:], op[:])
                        else:
                            nc.vector.tensor_copy(ot[:], op[:])
                        nc.sync.dma_start(
                            out=out_d.ap()[ts, co * QTILE : (co + 1) * QTILE],
                            in_=ot[:],
                        )

            # ---- software-pipelined emission ----
            stage1(0)
            stage1(1)
            attention(0)
            stage1(2)
            attention(1)
            stage1(3)
            outproj(0)
            attention(2)
            outproj(1)
            attention(3)
            outproj(2)
            outproj(3)
    nc.finalize()
    return nc


def _rope_tables():
    freqs = 1.0 / (10000.0 ** (np.arange(0, D, 2, dtype=np.float64) / D))  # [32]
    grid = np.arange(T, dtype=np.float64)[:, None] * freqs[None, :]  # [T, 32]
    cos = np.cos(grid)
    sin = np.sin(grid)
    # row d uses freq d//2; sin sign: + for even d, - for odd d
    cos_b = np.repeat(cos.T, 2, axis=0)  # [64, T]
    sin_b = np.repeat(sin.T, 2, axis=0)
    sin_b[1::2] *= -1.0
    cos_hd = np.tile(cos_b, (2, 1)).astype(np.float32)  # [128, T]
    sin_hd = np.tile(sin_b, (2, 1)).astype(np.float32)
    return np.ascontiguousarray(cos_hd), np.ascontiguousarray(sin_hd)


def _pswap():
    p = np.zeros((128, 128), dtype=np.float32)
    idx = np.arange(0, 128, 2)
    p[idx, idx + 1] = 1.0
    p[idx + 1, idx] = 1.0
    return p.astype(BF)


def kernel(x, wq, wk, wv, wo):
    if "nc" not in _CACHE:
        _CACHE["nc"] = build()
    nc = _CACHE["nc"]

    cos_hd, sin_hd = _rope_tables()
    psw = _pswap()
    x_bf = np.ascontiguousarray(x, dtype=np.float32).astype(BF)
    core_ids = list(range(NCORES))
    in_maps = []
    for c in core_ids:
        b, hh = c // 2, c % 2
        sl = slice(hh * HD, (hh + 1) * HD)
        in_maps.append(
            {
                "x": np.ascontiguousarray(x_bf[b]),
                "wqt": np.ascontiguousarray(wq[sl, :].T.astype(BF)),
                "wkt": np.ascontiguousarray(wk[sl, :].T.astype(BF)),
                "wvt": np.ascontiguousarray(wv[sl, :].T.astype(BF)),
                "wot": np.ascontiguousarray(wo[:, sl].T.astype(BF)),
                "cosb": cos_hd,
                "sinb": sin_hd,
                "pswap": psw,
            }
        )
    try:
        res = run_bass_kernel_spmd(nc, in_maps, core_ids).results
    except Exception:
        # transient NRT/device hiccup: retry once
        res = run_bass_kernel_spmd(nc, in_maps, core_ids).results
    out = np.zeros((B, T, C), dtype=np.float32)
    for c in core_ids:
        out[c // 2] += res[c]["out"]
    return out
